# revision 1
# baseline (speedup 1.0000x reference)
"""Trainium2 Bass kernel for nn_Entropy (histogram_binning): per-pixel Shannon
entropy of a 5x5-window KDE histogram over 256 intensity bins.

Math (validated in f32 vs reference):
  k(x,b) = sigmoid'(10(x-b)) = 0.25*(1 - tanh^2(5x-5b))   [exact identity]
  q[h,w,b] = 5x5 window sum of k;  S = sum_b q;  p = q/(S+EPS)
  out = -sum_b p*ln(p+EPS) = -r * sum_b q*ln(r*q+EPS),  r = 1/(S+EPS)
  S comes analytically per pixel from 5 taps of the KDE kernel around
  frac(x) (range-masked), then a tiny 5x5 window sum.

Layout per (image, bin-half) stripe: partitions = h (96), free = (w, b).
  - d' = 5x - 5b on TensorE: K=9 matmuls; stationary = [5*x^T(8 w-rows);
    ones], moving = tiny shipped selector constant.
  - tanh on ScalarE (evacuates PSUM); k = 0.25 - 0.25 t^2 on VectorE.
  - H-window: banded-matrix matmul (TensorE) -> PSUM, evacuated by
    ScalarE into a w-inner padded stripe [b-block: 3 zero pads + 96 w].
  - W-window: one in-place VectorE prefix scan per stripe over the padded
    row; q[w,b] = P[99b+w+5] - P[99b+w] (pads absorb all edges).
  - backend per w: L = ln(r*q + EPS) on ScalarE (per-partition scale AP),
    e = q*L and QL = sum_b(e) on VectorE; E = -r*QL.

Stripes are software-pipelined (3 stripe buffers) so image i+1's front end
overlaps image i's backend. Sharding: B*C = 24 images, 3 per core across 8
cores; no collectives. Self-contained; compiled once per process.
"""

import sys

sys.path.insert(0, "/opt/trn_rl_repo")

import numpy as np

H = 96
W = 96
NB = 256
NBH = 128         # bins per stripe (half)
NIMG = 3
NCORES = 8
EPS = 1e-10
ZB = 99           # per-bin block in a stripe: 3 zero pads + 96 w cols
WQ = 8            # w rows per stationary group
NG = W // WQ      # 12 groups

_CACHE = {}


def _build_consts():
    # selector constants per bin-half: [9, WQ*NBH]; rows j=0..7 mark w-offset
    # j over that bin-block; row 8 = -5*b
    crhs = []
    for half in range(2):
        c = np.zeros((9, WQ * NBH), dtype=np.float32)
        for j in range(WQ):
            c[j, j * NBH:(j + 1) * NBH] = 1.0
        b = np.arange(NBH, dtype=np.float32) + half * NBH
        c[8, :] = np.tile(-5.0 * b, WQ)
        crhs.append(c)
    hh = np.arange(H)
    band = (np.abs(hh[:, None] - hh[None, :]) <= 2).astype(np.float32)
    return crhs[0], crhs[1], band


def _emit_kernel(nc, tc, ctx, ins, outs):
    from concourse import mybir

    f32 = mybir.dt.float32
    i32 = mybir.dt.int32
    AF = mybir.ActivationFunctionType
    OP = mybir.AluOpType

    x_d, xt_d, crhs0_d, crhs1_d, band_d = ins
    (ent_d,) = outs
    NW = NIMG * W

    consts = ctx.enter_context(tc.tile_pool(name="consts", bufs=1))
    stripes = ctx.enter_context(tc.tile_pool(name="stripes", bufs=2))
    sm = ctx.enter_context(tc.tile_pool(name="sm", bufs=1))
    chunks = ctx.enter_context(tc.tile_pool(name="chunks", bufs=2))
    psum = ctx.enter_context(tc.tile_pool(name="psum", bufs=4, space="PSUM"))

    # ---- constants / inputs ----
    crhs_sb = []
    for half, cd in ((0, crhs0_d), (1, crhs1_d)):
        t = consts.tile([73, WQ * NBH], f32, tag=f"crhs{half}")
        for k3 in range(3):
            nc.sync.dma_start(t[32 * k3:32 * k3 + 9, :], cd[:])
        crhs_sb.append(t)
    band_sb = consts.tile([H, H], f32)
    nc.sync.dma_start(band_sb[:], band_d[:])

    xall = consts.tile([H, NW], f32)
    xtall = consts.tile([W, NIMG * H], f32)
    for i in range(NIMG):
        nc.sync.dma_start(xall[:, i * W:(i + 1) * W], x_d[i])
        nc.sync.dma_start(xtall[:, i * H:(i + 1) * H], xt_d[i])

    ones_sb = consts.tile([1, NIMG * H], f32)
    nc.vector.memset(ones_sb[:], 1.0)
    xt5_all = consts.tile([W, NIMG * H], f32)
    nc.vector.tensor_scalar(xt5_all[:], xtall[:], 5.0, None, op0=OP.mult)
    # stationary groups [9 rows: 5*xT(8 w) ; ones], 3 per tile at bases 0/32/64
    xt9g = []
    for tg in range(4):
        gt = consts.tile([73, NIMG * H], f32, tag=f"xt9g{tg}")
        for k3 in range(3):
            g = tg * 3 + k3
            base = 32 * k3
            nc.sync.dma_start(gt[base:base + 8, :], xt5_all[8 * g:8 * g + 8, :])
            nc.sync.dma_start(gt[base + 8:base + 9, :], ones_sb[:])
            xt9g.append(gt[base:base + 9])

    bias_tiles = {}

    def bias_ap(val):
        if val not in bias_tiles:
            t = consts.tile([H, 1], f32, tag=f"bias{val}")
            nc.vector.memset(t[:], val)
            bias_tiles[val] = t
        return bias_tiles[val][:]

    # =====================  S path (tiny, [96, 288])  =====================
    ni = sm.tile([H, NW], i32)
    nc.vector.tensor_copy(ni[:], xall[:])
    nf = sm.tile([H, NW], f32)
    nc.vector.tensor_copy(nf[:], ni[:])
    u = sm.tile([H, NW], f32)
    nc.vector.tensor_tensor(u[:], xall[:], nf[:], op=OP.subtract)
    taps = (-2, -1, 0, 1, 2)
    sq = {}
    for o in taps:
        v = sm.tile([H, NW], f32, tag=f"v{o}")
        nc.scalar.activation(v[:], u[:], AF.Tanh, bias=bias_ap(-5.0 * o), scale=5.0)
        s2 = sm.tile([H, NW], f32, tag=f"sq{o}")
        nc.scalar.activation(s2[:], v[:], AF.Square)
        sq[o] = s2
    masks = {}
    for o in taps:
        if o == 0:
            continue
        m = sm.tile([H, NW], f32, tag=f"m{o}")
        if o < 0:
            nc.vector.tensor_scalar(m[:], nf[:], float(-o), None, op0=OP.is_ge)
        else:
            nc.vector.tensor_scalar(m[:], nf[:], float(255 - o), None, op0=OP.is_le)
        masks[o] = m
    cnt = sm.tile([H, NW], f32)
    nc.vector.tensor_tensor(cnt[:], masks[-2][:], masks[-1][:], op=OP.add)
    nc.vector.tensor_tensor(cnt[:], cnt[:], masks[1][:], op=OP.add)
    nc.vector.tensor_tensor(cnt[:], cnt[:], masks[2][:], op=OP.add)
    nc.vector.tensor_scalar(cnt[:], cnt[:], 1.0, None, op0=OP.add)
    ssum = sm.tile([H, NW], f32)
    nc.vector.tensor_copy(ssum[:], sq[0][:])
    for o in (-2, -1, 1, 2):
        t_m = sm.tile([H, NW], f32, tag=f"tm{o}")
        nc.vector.tensor_tensor(t_m[:], masks[o][:], sq[o][:], op=OP.mult)
        nc.vector.tensor_tensor(ssum[:], ssum[:], t_m[:], op=OP.add)
    spix = sm.tile([H, NW], f32)
    nc.vector.tensor_tensor(spix[:], cnt[:], ssum[:], op=OP.subtract)
    nc.vector.tensor_scalar(spix[:], spix[:], 0.25, None, op0=OP.mult)
    ps_s = psum.tile([H, 1024], f32, tag="ps")
    nc.tensor.matmul(ps_s[:, 0:NW], band_sb[:], spix[:], start=True, stop=True)
    sh = sm.tile([H, NW], f32)
    nc.scalar.copy(sh[:], ps_s[:, 0:NW])
    shp = sm.tile([H, NIMG, W + 4], f32)
    nc.vector.memset(shp[:], 0.0)
    for i in range(NIMG):
        nc.vector.tensor_copy(shp[:, i, 2:2 + W], sh[:, i * W:(i + 1) * W])
    swin = sm.tile([H, NIMG, W], f32)
    nc.vector.tensor_tensor(swin[:], shp[:, :, 0:W], shp[:, :, 1:1 + W], op=OP.add)
    for j in (2, 3, 4):
        nc.vector.tensor_tensor(swin[:], swin[:], shp[:, :, j:j + W], op=OP.add)
    rtile = sm.tile([H, NW], f32)
    sw_flat = swin[:].rearrange("p a b -> p (a b)")
    nc.vector.tensor_scalar(rtile[:], sw_flat, EPS, None, op0=OP.add)
    nc.vector.reciprocal(rtile[:], rtile[:])

    # =====================  main path: per (image, bin-half) stripe  ========
    QL = sm.tile([H, NW], f32)
    stripe_store = {}

    def emit_front(i, half):
        qh = stripes.tile([H, NBH * ZB + 8], f32, tag="qh")
        qh3 = qh[:, 0:NBH * ZB].rearrange("p (b z) -> p b z", z=ZB)
        nc.vector.memset(qh3[:, :, 0:3], 0.0)
        nc.vector.memset(qh[:, NBH * ZB:], 0.0)

        for c in range(NG // 2):  # chunks of 2 w-groups = [96, 2048] cols
            pd = psum.tile([H, 1024], f32, tag="ps")
            pd2 = psum.tile([H, 1024], f32, tag="ps")
            for piece, pt in ((0, pd), (1, pd2)):
                g = 2 * c + piece
                base = 32 * (g % 3)
                nc.tensor.matmul(
                    pt[:, 0:512],
                    xt9g[g][:, i * H:(i + 1) * H],
                    crhs_sb[half][base:base + 9, 0:512],
                    start=True, stop=True,
                )
                nc.tensor.matmul(
                    pt[:, 512:1024],
                    xt9g[g][:, i * H:(i + 1) * H],
                    crhs_sb[half][base:base + 9, 512:1024],
                    start=True, stop=True,
                )
            tt = chunks.tile([H, 2048], f32, tag="t")
            nc.scalar.activation(tt[:, 0:1024], pd[:], AF.Tanh)
            nc.scalar.activation(tt[:, 1024:2048], pd2[:], AF.Tanh)
            kk = chunks.tile([H, 2048], f32, tag="k")
            nc.vector.tensor_tensor(kk[:], tt[:], tt[:], op=OP.mult)
            nc.vector.tensor_scalar(kk[:], kk[:], -0.25, 0.25, op0=OP.mult, op1=OP.add)
            for piece in range(2):
                ph = psum.tile([H, 1024], f32, tag="ps")
                for pp in range(2):
                    nc.tensor.matmul(
                        ph[:, pp * 512:(pp + 1) * 512],
                        band_sb[:],
                        kk[:, piece * 1024 + pp * 512:piece * 1024 + (pp + 1) * 512],
                        start=True, stop=True,
                    )
                # evac: chunk piece covers w-group g = 2c+piece (8 w), all bins
                g = 2 * c + piece
                dst = qh3[:, :, 3 + 8 * g:3 + 8 * g + 8].transpose([0, 2, 1])
                nc.scalar.copy(dst, ph[:].rearrange("p (w b) -> p w b", b=NBH))

        nc.vector.tensor_tensor_scan(
            qh[:], qh[:], qh[:], 0.0, op0=OP.add, op1=OP.bypass
        )
        stripe_store[(i, half)] = (qh, qh3)

    def emit_backend(i):
        qhs = [stripe_store.pop((i, 0)), stripe_store.pop((i, 1))]
        for wc in range(W // 4):
            w0 = 4 * wc
            qt = chunks.tile([H, 4, NB], f32, tag="q")
            for half, (qh, qh3) in enumerate(qhs):
                if w0 + 9 <= ZB:
                    hi = qh3[:, :, w0 + 5:w0 + 9].transpose([0, 2, 1])
                    lo = qh3[:, :, w0:w0 + 4].transpose([0, 2, 1])
                    nc.vector.tensor_tensor(
                        qt[:, :, half * NBH:(half + 1) * NBH], hi, lo,
                        op=OP.subtract,
                    )
                else:
                    for wi in range(4):
                        nc.vector.tensor_tensor(
                            qt[:, wi, half * NBH:(half + 1) * NBH],
                            qh[:, w0 + 5 + wi::ZB][:, 0:NBH],
                            qh[:, w0 + wi::ZB][:, 0:NBH],
                            op=OP.subtract,
                        )
            ltile = chunks.tile([H, 1024], f32, tag="L")
            for j in range(4):
                w = w0 + j
                rcol = rtile[:, i * W + w:i * W + w + 1]
                nc.scalar.activation(
                    ltile[:, j * 256:(j + 1) * 256],
                    qt[:, j, :],
                    AF.Ln,
                    bias=bias_ap(EPS),
                    scale=rcol,
                )
            l3 = ltile[:].rearrange("p (a b) -> p a b", b=NB)
            nc.vector.tensor_tensor(l3, qt[:], l3, op=OP.mult)
            nc.vector.tensor_reduce(
                QL[:, i * W + w0:i * W + w0 + 4],
                l3,
                axis=mybir.AxisListType.X,
                op=OP.add,
            )

    emit_front(0, 0)
    emit_front(0, 1)
    emit_front(1, 0)
    emit_backend(0)
    emit_front(1, 1)
    emit_front(2, 0)
    emit_backend(1)
    emit_front(2, 1)
    emit_backend(2)

    # E = -(r * QL) ; write out
    ent = sm.tile([H, NW], f32)
    nc.vector.tensor_tensor(ent[:], rtile[:], QL[:], op=OP.mult)
    nc.vector.tensor_scalar(ent[:], ent[:], -1.0, None, op0=OP.mult)
    for i in range(NIMG):
        nc.sync.dma_start(ent_d[i], ent[:, i * W:(i + 1) * W])


def _get_compiled():
    if "nc" in _CACHE:
        return _CACHE["nc"]
    from contextlib import ExitStack

    import concourse.tile as tile
    from concourse import bacc, mybir

    f32 = mybir.dt.float32
    nc = bacc.Bacc("TRN2", target_bir_lowering=False, debug=False)
    x_d = nc.dram_tensor("x_sh", [NIMG, H, W], f32, kind="ExternalInput").ap()
    xt_d = nc.dram_tensor("xt_sh", [NIMG, W, H], f32, kind="ExternalInput").ap()
    crhs0_d = nc.dram_tensor("crhs0", [9, WQ * NBH], f32, kind="ExternalInput").ap()
    crhs1_d = nc.dram_tensor("crhs1", [9, WQ * NBH], f32, kind="ExternalInput").ap()
    band_d = nc.dram_tensor("bandh", [H, H], f32, kind="ExternalInput").ap()
    ent_d = nc.dram_tensor("ent", [NIMG, H, W], f32, kind="ExternalOutput").ap()

    with tile.TileContext(nc) as tc:
        with ExitStack() as ctx:
            _emit_kernel(
                nc, tc, ctx, (x_d, xt_d, crhs0_d, crhs1_d, band_d), (ent_d,)
            )
    nc.compile()
    _CACHE["nc"] = nc
    return nc


def make_in_maps(x):
    """x: full [8, 3, 96, 96] -> list of 8 per-core input dicts."""
    x = np.ascontiguousarray(np.asarray(x, dtype=np.float32))
    imgs = x.reshape(NCORES * NIMG, H, W)
    crhs0, crhs1, band = _build_consts()
    in_maps = []
    for c in range(NCORES):
        sh = np.ascontiguousarray(imgs[c * NIMG:(c + 1) * NIMG])
        in_maps.append(
            {
                "x_sh": sh,
                "xt_sh": np.ascontiguousarray(sh.transpose(0, 2, 1)),
                "crhs0": crhs0,
                "crhs1": crhs1,
                "bandh": band,
            }
        )
    return in_maps


def kernel(x):
    """Full inputs in, full outputs out. x: [8, 3, 96, 96] f32."""
    from concourse.bass_utils import run_bass_kernel_spmd

    nc = _get_compiled()
    in_maps = make_in_maps(x)
    res = run_bass_kernel_spmd(nc, in_maps, list(range(NCORES)))
    out = np.stack([res.results[c]["ent"] for c in range(NCORES)])
    return out.reshape(8, 3, H, W).astype(np.float32)



# revision 4
# speedup vs baseline: 2.6985x; 2.6985x over previous
"""Trainium2 Bass kernel for nn_Entropy (histogram_binning): per-pixel Shannon
entropy of a 5x5-window KDE histogram over 256 intensity bins.

Math (validated in numpy to 2.1e-3 max rel err vs f64 oracle):
  k(x,b) = sigmoid'(10(x-b)) = 0.25*(1 - tanh^2(5(x-b)))
  Scale factors cancel in p = q/S, so we use m = 1 - t^2 directly.
  q[h,w,b] = 5x5 window sum of m = cnt(h,w) - winsum5x5(t^2)
  E = -sum_b p ln p = ln(S) - (sum_b q ln(q+EPS))/S,  S = sum_b q
  S comes analytically per pixel from 5 taps of 1-tanh^2 around frac(x)
  (range-masked), then a 5x5 window sum (H via matmul, W via shifted adds).

Pipeline per image (layout: partitions = h, free = (w-block, 256 bins)):
  MM1 (TensorE, fp16): d = 5u + 5n - 5b via stationary [4x 5u^T; 4x 5n^T;
    ones] and a delta-selector moving constant -> PSUM f32. The u/n split
    keeps d exact in fp16 where |d| is small (n,b integers are fp16-exact).
  tanh (ScalarE): PSUM -> SBUF fp16.
  square (DVE, fp16 2x): t^2 -> stripe [97, 100 blocks x 256] with 2+2
    zero-pad w-blocks; row 96 = in-range-w indicator (cw row).
  MM2 (TensorE, fp16, 5 shifted taps accumulated in PSUM): stationary
    [-band(96x96); ch(h)] x stripe -> q = cnt - winsum5x5(t^2) directly in
    PSUM. No scan, no transpose, no separate W pass.
  backend: Ln(q+EPS) (ScalarE, PSUM src), then tensor_tensor_reduce
    (DVE) computes sum_b q*ln per w column -> QL.
  E = lnS - QL/S on [96, 288] tiles.

ScalarE activations are batched per image phase (tanh batch / Ln batch) to
avoid activation-table thrash. Sharding: B*C = 24 images, 3 per core on 8
cores; no collectives. Self-contained; compiled once per process.
"""

import sys

sys.path.insert(0, "/opt/trn_rl_repo")

import numpy as np

H = 96
W = 96
NB = 256
NIMG = 3
NCORES = 8
EPS = 1e-10
WG = 4                 # w's per MM1 group
NG = W // WG           # 24 MM1 groups per image
NBLK = W + 4           # stripe w-blocks incl 2+2 pads
SCOLS = NBLK * NB      # 25600 stripe cols
NW = NIMG * W          # 288
USE_TTR = False        # tensor_tensor_reduce backend vs TT+reduce

_CACHE = {}


def _build_consts():
    # MM1 moving selector [9, WG*NB] fp16: col (w', b) picks stationary rows
    # w' (5u) and 4+w' (5n), plus -5b via the ones row.
    sel = np.zeros((9, WG * NB), dtype=np.float32)
    b = np.arange(NB, dtype=np.float32)
    for j in range(WG):
        sel[j, j * NB:(j + 1) * NB] = 1.0
        sel[4 + j, j * NB:(j + 1) * NB] = 1.0
    sel[8, :] = np.tile(-5.0 * b, WG)

    hh = np.arange(H)
    band = (np.abs(hh[:, None] - hh[None, :]) <= 2).astype(np.float32)
    bandch = np.zeros((H + 1, H), dtype=np.float32)
    bandch[:H, :] = -band
    bandch[H, :] = band.sum(0)  # ch(h) in {3,4,5}
    return sel.astype(np.float16), bandch.astype(np.float16)


def _emit_kernel(nc, tc, ctx, ins, outs):
    from concourse import mybir

    f32 = mybir.dt.float32
    f16 = mybir.dt.float16
    AF = mybir.ActivationFunctionType
    OP = mybir.AluOpType

    u_d, nf_d, st_d, sel_d, bandch_d, padz_d, ones_d = ins
    (ent_d,) = outs

    consts = ctx.enter_context(tc.tile_pool(name="consts", bufs=1))
    sm = ctx.enter_context(tc.tile_pool(name="sm", bufs=1))
    tpool = ctx.enter_context(tc.tile_pool(name="tpool", bufs=2))
    lpool = ctx.enter_context(tc.tile_pool(name="lpool", bufs=2))
    pmm1 = ctx.enter_context(tc.tile_pool(name="pmm1", bufs=2, space="PSUM"))
    pmm2 = ctx.enter_context(tc.tile_pool(name="pmm2", bufs=3, space="PSUM"))

    # ---- inputs / consts ----
    u_sb = consts.tile([H, NW], f32)
    nf_sb = consts.tile([H, NW], f32)
    st_sb = consts.tile([9, NIMG * NG * H], f16)
    sel_sb = consts.tile([9, WG * NB], f16)
    bandch_sb = consts.tile([H + 1, H], f16)
    nc.sync.dma_start(u_sb[:], u_d[:])
    nc.sync.dma_start(nf_sb[:], nf_d[:])
    nc.sync.dma_start(st_sb[:], st_d[:])
    nc.sync.dma_start(sel_sb[:], sel_d[:])
    nc.sync.dma_start(bandch_sb[:], bandch_d[:])

    # two persistent stripes; pads + cw-indicator row initialized via DMA
    stripes = []
    for tag in ("stripeA", "stripeB"):
        s = consts.tile([H + 1, SCOLS], f16, tag=tag)
        nc.sync.dma_start(s[:, 0:2 * NB], padz_d[:])
        nc.sync.dma_start(s[:, SCOLS - 2 * NB:SCOLS], padz_d[:])
        nc.sync.dma_start(s[H:H + 1, 2 * NB:SCOLS - 2 * NB], ones_d[:])
        stripes.append(s)

    bias_tiles = {}

    def bias_ap(val):
        if val not in bias_tiles:
            t = consts.tile([H, 1], f32, tag=f"bias{val}")
            nc.vector.memset(t[:], val)
            bias_tiles[val] = t
        return bias_tiles[val][:]

    # =====================  S path (tiny, [96, 288])  =====================
    taps = (-2, -1, 0, 1, 2)
    sq = {}
    for o in taps:
        v = sm.tile([H, NW], f32, tag=f"v{o}")
        nc.scalar.activation(v[:], u_sb[:], AF.Tanh, bias=bias_ap(-5.0 * o), scale=5.0)
        s2 = sm.tile([H, NW], f32, tag=f"sq{o}")
        nc.scalar.activation(s2[:], v[:], AF.Square)
        sq[o] = s2
    masks = {}
    for o in taps:
        if o == 0:
            continue
        m = sm.tile([H, NW], f32, tag=f"m{o}")
        if o < 0:
            nc.vector.tensor_scalar(m[:], nf_sb[:], float(-o), None, op0=OP.is_ge)
        else:
            nc.vector.tensor_scalar(m[:], nf_sb[:], float(255 - o), None, op0=OP.is_le)
        masks[o] = m
    cnt = sm.tile([H, NW], f32)
    nc.vector.tensor_tensor(cnt[:], masks[-2][:], masks[-1][:], op=OP.add)
    nc.vector.tensor_tensor(cnt[:], cnt[:], masks[1][:], op=OP.add)
    nc.vector.tensor_tensor(cnt[:], cnt[:], masks[2][:], op=OP.add)
    nc.vector.tensor_scalar(cnt[:], cnt[:], 1.0, None, op0=OP.add)
    ssum = sm.tile([H, NW], f32)
    nc.vector.tensor_copy(ssum[:], sq[0][:])
    for o in (-2, -1, 1, 2):
        t_m = sm.tile([H, NW], f32, tag=f"tm{o}")
        nc.vector.tensor_tensor(t_m[:], masks[o][:], sq[o][:], op=OP.mult)
        nc.vector.tensor_tensor(ssum[:], ssum[:], t_m[:], op=OP.add)
    # spix = cnt - ssum; build negated fp16 copy with a zero 97th row for the
    # -band/ch stationary
    spix = sm.tile([H, NW], f32)
    nc.vector.tensor_tensor(spix[:], cnt[:], ssum[:], op=OP.subtract)
    sneg = sm.tile([H + 1, NW], f16)
    nc.vector.memset(sneg[H:H + 1, :], 0.0)
    nc.vector.tensor_scalar(sneg[0:H, :], spix[:], -1.0, None, op0=OP.mult)
    ps_s = pmm2.tile([H, 512], f32, tag="ps2")
    nc.tensor.matmul(ps_s[:, 0:NW], bandch_sb[:], sneg[:], start=True, stop=True)
    sh = sm.tile([H, NW], f32)
    nc.scalar.copy(sh[:], ps_s[:, 0:NW])
    shp = sm.tile([H, NIMG, W + 4], f32)
    nc.vector.memset(shp[:], 0.0)
    for i in range(NIMG):
        nc.vector.tensor_copy(shp[:, i, 2:2 + W], sh[:, i * W:(i + 1) * W])
    swin = sm.tile([H, NIMG, W], f32)
    nc.vector.tensor_tensor(swin[:], shp[:, :, 0:W], shp[:, :, 1:1 + W], op=OP.add)
    for j in (2, 3, 4):
        nc.vector.tensor_tensor(swin[:], swin[:], shp[:, :, j:j + W], op=OP.add)
    sw_flat = swin[:].rearrange("p a b -> p (a b)")
    rinv = sm.tile([H, NW], f32)
    nc.vector.tensor_scalar(rinv[:], sw_flat, EPS, None, op0=OP.add)
    nc.vector.reciprocal(rinv[:], rinv[:])
    lnS = sm.tile([H, NW], f32)

    # =====================  main loop  =====================
    QL = sm.tile([H, NW], f32)
    dummy = sm.tile([H, 1], f32)

    def front(i):
        stripe = stripes[i % 2]
        for g in range(NG):
            gi = i * NG + g
            ps1 = pmm1.tile([H, 1024], f32, tag="ps1")
            stat = st_sb[:, gi * H:(gi + 1) * H]
            nc.tensor.matmul(ps1[:, 0:512], stat, sel_sb[:, 0:512],
                             start=True, stop=True)
            nc.tensor.matmul(ps1[:, 512:1024], stat, sel_sb[:, 512:1024],
                             start=True, stop=True)
            t = tpool.tile([H, 1024], f16, tag="t")
            nc.scalar.activation(t[:], ps1[:], AF.Tanh)
            dst = stripe[0:H, (WG * g + 2) * NB:(WG * g + 2 + WG) * NB]
            nc.vector.tensor_tensor(dst, t[:], t[:], op=OP.mult)

    def back(i):
        stripe = stripes[i % 2]
        if i == 0:
            nc.scalar.activation(lnS[:], sw_flat, AF.Ln)
        for c in range(W // 2):
            ps2 = pmm2.tile([H, 512], f32, tag="ps2")
            for tap, dw in enumerate((-2, -1, 0, 1, 2)):
                blk = 2 * c + 2 + dw
                nc.tensor.matmul(
                    ps2[:],
                    bandch_sb[:],
                    stripe[:, blk * NB:(blk + 2) * NB],
                    start=(tap == 0),
                    stop=(tap == 4),
                )
            L = lpool.tile([H, 512], f32, tag="L")
            nc.scalar.activation(L[:], ps2[:], AF.Ln, bias=bias_ap(EPS))
            if USE_TTR:
                for j in range(2):
                    w = 2 * c + j
                    nc.vector.tensor_tensor_reduce(
                        dummy.broadcast_to((H, NB)),
                        ps2[:, j * NB:(j + 1) * NB],
                        L[:, j * NB:(j + 1) * NB],
                        scale=1.0,
                        scalar=0.0,
                        op0=OP.mult,
                        op1=OP.add,
                        accum_out=QL[:, i * W + w:i * W + w + 1],
                    )
            else:
                w = 2 * c
                nc.vector.tensor_tensor(L[:], ps2[:], L[:], op=OP.mult)
                l3 = L[:].rearrange("p (a b) -> p a b", b=NB)
                nc.vector.tensor_reduce(
                    QL[:, i * W + w:i * W + w + 2],
                    l3,
                    axis=mybir.AxisListType.X,
                    op=OP.add,
                )

    for i in range(NIMG):
        front(i)
        back(i)

    # E = lnS - QL / S
    ent = sm.tile([H, NW], f32)
    nc.vector.tensor_tensor(ent[:], QL[:], rinv[:], op=OP.mult)
    nc.vector.tensor_tensor(ent[:], lnS[:], ent[:], op=OP.subtract)
    for i in range(NIMG):
        nc.sync.dma_start(ent_d[i], ent[:, i * W:(i + 1) * W])


def _get_compiled():
    if "nc" in _CACHE:
        return _CACHE["nc"]
    from contextlib import ExitStack

    import concourse.tile as tile
    from concourse import bacc, mybir

    f32 = mybir.dt.float32
    f16 = mybir.dt.float16
    nc = bacc.Bacc("TRN2", target_bir_lowering=False, debug=False)
    u_d = nc.dram_tensor("u_sh", [H, NW], f32, kind="ExternalInput").ap()
    nf_d = nc.dram_tensor("nf_sh", [H, NW], f32, kind="ExternalInput").ap()
    st_d = nc.dram_tensor("st_sh", [9, NIMG * NG * H], f16, kind="ExternalInput").ap()
    sel_d = nc.dram_tensor("sel", [9, WG * NB], f16, kind="ExternalInput").ap()
    bandch_d = nc.dram_tensor("bandch", [H + 1, H], f16, kind="ExternalInput").ap()
    padz_d = nc.dram_tensor("padz", [H + 1, 2 * NB], f16, kind="ExternalInput").ap()
    ones_d = nc.dram_tensor("onesmid", [1, W * NB], f16, kind="ExternalInput").ap()
    ent_d = nc.dram_tensor("ent", [NIMG, H, W], f32, kind="ExternalOutput").ap()

    with tile.TileContext(nc) as tc:
        with ExitStack() as ctx:
            _emit_kernel(
                nc, tc, ctx,
                (u_d, nf_d, st_d, sel_d, bandch_d, padz_d, ones_d),
                (ent_d,),
            )
    nc.compile()
    _CACHE["nc"] = nc
    return nc


def make_in_maps(x):
    """x: full [8, 3, 96, 96] -> list of 8 per-core input dicts."""
    x = np.ascontiguousarray(np.asarray(x, dtype=np.float32))
    imgs = x.reshape(NCORES * NIMG, H, W)
    sel, bandch = _build_consts()
    padz = np.zeros((H + 1, 2 * NB), dtype=np.float16)
    onesmid = np.ones((1, W * NB), dtype=np.float16)
    in_maps = []
    for c in range(NCORES):
        sh = imgs[c * NIMG:(c + 1) * NIMG]            # [3, 96, 96]
        n = np.trunc(sh)
        u = sh - n
        # [h, i*96+w] layouts for the S path
        u_all = np.ascontiguousarray(u.transpose(1, 0, 2).reshape(H, NW))
        nf_all = np.ascontiguousarray(n.transpose(1, 0, 2).reshape(H, NW))
        # stationary groups: [9, 3*24*96] fp16
        u5t = (5.0 * u).transpose(0, 2, 1).astype(np.float16)   # [3, 96w, 96h]
        n5t = (5.0 * n).transpose(0, 2, 1).astype(np.float16)
        st = np.empty((9, NIMG * NG * H), dtype=np.float16)
        st[8, :] = 1.0
        for i in range(NIMG):
            for g in range(NG):
                col = (i * NG + g) * H
                st[0:WG, col:col + H] = u5t[i, WG * g:WG * g + WG, :]
                st[WG:2 * WG, col:col + H] = n5t[i, WG * g:WG * g + WG, :]
        in_maps.append(
            {
                "u_sh": u_all,
                "nf_sh": nf_all,
                "st_sh": st,
                "sel": sel,
                "bandch": bandch,
                "padz": padz,
                "onesmid": onesmid,
            }
        )
    return in_maps


def kernel(x):
    """Full inputs in, full outputs out. x: [8, 3, 96, 96] f32."""
    from concourse.bass_utils import run_bass_kernel_spmd

    nc = _get_compiled()
    in_maps = make_in_maps(x)
    res = run_bass_kernel_spmd(nc, in_maps, list(range(NCORES)))
    out = np.stack([res.results[c]["ent"] for c in range(NCORES)])
    return out.reshape(8, 3, H, W).astype(np.float32)


# revision 7
# speedup vs baseline: 3.3472x; 1.2404x over previous
"""Trainium2 Bass kernel for nn_Entropy (histogram_binning): per-pixel Shannon
entropy of a 5x5-window KDE histogram over 256 intensity bins.

Math (validated in numpy to 2.1e-3 max rel err vs f64 oracle):
  k(x,b) = sigmoid'(10(x-b)) = 0.25*(1 - tanh^2(5(x-b)))
  Scale factors cancel in p = q/S, so we use m = 1 - t^2 directly.
  q[h,w,b] = 5x5 window sum of m = cnt(h,w) - winsum5x5(t^2)
  E = -sum_b p ln p = ln(S) - (sum_b q ln(q+EPS))/S,  S = sum_b q
  S comes analytically per pixel from 5 taps of 1-tanh^2 around frac(x)
  (range-masked), then a 5x5 window sum (H via matmul, W via shifted adds).

Pipeline per image (layout: partitions = h, free = (w-block, 256 bins)):
  MM1 (TensorE, fp16): d = 5u + 5n - 5b via stationary [4x 5u^T; 4x 5n^T;
    ones] and a delta-selector moving constant -> PSUM f32. The u/n split
    keeps d exact in fp16 where |d| is small (n,b integers are fp16-exact).
  tanh (ScalarE): PSUM -> SBUF fp16.
  square (DVE, fp16 2x): t^2 -> stripe [97, 100 blocks x 256] with 2+2
    zero-pad w-blocks; row 96 = in-range-w indicator (cw row).
  MM2 (TensorE, fp16, 5 shifted taps accumulated in PSUM): stationary
    [-band(96x96); ch(h)] x stripe -> q = cnt - winsum5x5(t^2) directly in
    PSUM. No scan, no transpose, no separate W pass.
  backend: Ln(q+EPS) (ScalarE, PSUM src), then tensor_tensor_reduce
    (DVE) computes sum_b q*ln per w column -> QL.
  E = lnS - QL/S on [96, 288] tiles.

ScalarE activations are batched per image phase (tanh batch / Ln batch) to
avoid activation-table thrash. Sharding: B*C = 24 images, 3 per core on 8
cores; no collectives. Self-contained; compiled once per process.
"""

import sys

sys.path.insert(0, "/opt/trn_rl_repo")

import numpy as np

H = 96
W = 96
NB = 256
NIMG = 3
NCORES = 8
EPS = 1e-10
WG = 4                 # w's per MM1 group
NG = W // WG           # 24 MM1 groups per image
NBLK = W + 4           # stripe w-blocks incl 2+2 pads
SCOLS = NBLK * NB      # 25600 stripe cols
NW = NIMG * W          # 288
BACKEND = "stt"        # "stt" | "ttr" | "reduce"

_CACHE = {}


def _build_consts():
    # MM1 moving selector [9, WG*NB] fp16: col (w', b) picks stationary rows
    # w' (5u) and 4+w' (5n), plus -5b via the ones row.
    sel = np.zeros((9, WG * NB), dtype=np.float32)
    b = np.arange(NB, dtype=np.float32)
    for j in range(WG):
        sel[j, j * NB:(j + 1) * NB] = 1.0
        sel[4 + j, j * NB:(j + 1) * NB] = 1.0
    sel[8, :] = np.tile(-5.0 * b, WG)

    hh = np.arange(H)
    band = (np.abs(hh[:, None] - hh[None, :]) <= 2).astype(np.float32)
    bandch = np.zeros((H + 1, H), dtype=np.float32)
    bandch[:H, :] = -band
    bandch[H, :] = band.sum(0)  # ch(h) in {3,4,5}
    return sel.astype(np.float16), bandch.astype(np.float16)


def _emit_kernel(nc, tc, ctx, ins, outs):
    from concourse import mybir

    f32 = mybir.dt.float32
    f16 = mybir.dt.float16
    AF = mybir.ActivationFunctionType
    OP = mybir.AluOpType

    u_d, nf_d, st_d, sel_d, bandch_d, padz_d, ones_d = ins
    (ent_d,) = outs

    consts = ctx.enter_context(tc.tile_pool(name="consts", bufs=1))
    sm = ctx.enter_context(tc.tile_pool(name="sm", bufs=1))
    tpool = ctx.enter_context(tc.tile_pool(name="tpool", bufs=2))
    lpool = ctx.enter_context(tc.tile_pool(name="lpool", bufs=2))
    pmm1 = ctx.enter_context(tc.tile_pool(name="pmm1", bufs=2, space="PSUM"))
    pmm2 = ctx.enter_context(tc.tile_pool(name="pmm2", bufs=4, space="PSUM"))

    # ---- inputs / consts ----
    u_sb = consts.tile([H, NW], f32)
    nf_sb = consts.tile([H, NW], f32)
    st_sb = consts.tile([9, NIMG * NG * H], f16)
    sel_sb = consts.tile([9, WG * NB], f16)
    bandch_sb = consts.tile([H + 1, H], f16)
    nc.sync.dma_start(u_sb[:], u_d[:])
    nc.sync.dma_start(nf_sb[:], nf_d[:])
    nc.sync.dma_start(st_sb[:], st_d[:])
    nc.sync.dma_start(sel_sb[:], sel_d[:])
    nc.sync.dma_start(bandch_sb[:], bandch_d[:])

    # two persistent stripes; pads + cw-indicator row initialized via DMA
    stripes = []
    for tag in ("stripeA", "stripeB"):
        s = consts.tile([H + 1, SCOLS], f16, tag=tag)
        nc.sync.dma_start(s[:, 0:2 * NB], padz_d[:])
        nc.sync.dma_start(s[:, SCOLS - 2 * NB:SCOLS], padz_d[:])
        nc.sync.dma_start(s[H:H + 1, 2 * NB:SCOLS - 2 * NB], ones_d[:])
        stripes.append(s)

    bias_tiles = {}

    def bias_ap(val):
        if val not in bias_tiles:
            t = consts.tile([H, 1], f32, tag=f"bias{val}")
            nc.vector.memset(t[:], val)
            bias_tiles[val] = t
        return bias_tiles[val][:]

    # =====================  S path (tiny, [96, 288])  =====================
    taps = (-2, -1, 0, 1, 2)
    sq = {}
    for o in taps:
        v = sm.tile([H, NW], f32, tag=f"v{o}")
        nc.scalar.activation(v[:], u_sb[:], AF.Tanh, bias=bias_ap(-5.0 * o), scale=5.0)
        s2 = sm.tile([H, NW], f32, tag=f"sq{o}")
        nc.scalar.activation(s2[:], v[:], AF.Square)
        sq[o] = s2
    masks = {}
    for o in taps:
        if o == 0:
            continue
        m = sm.tile([H, NW], f32, tag=f"m{o}")
        if o < 0:
            nc.vector.tensor_scalar(m[:], nf_sb[:], float(-o), None, op0=OP.is_ge)
        else:
            nc.vector.tensor_scalar(m[:], nf_sb[:], float(255 - o), None, op0=OP.is_le)
        masks[o] = m
    cnt = sm.tile([H, NW], f32)
    nc.vector.tensor_tensor(cnt[:], masks[-2][:], masks[-1][:], op=OP.add)
    nc.vector.tensor_tensor(cnt[:], cnt[:], masks[1][:], op=OP.add)
    nc.vector.tensor_tensor(cnt[:], cnt[:], masks[2][:], op=OP.add)
    nc.vector.tensor_scalar(cnt[:], cnt[:], 1.0, None, op0=OP.add)
    ssum = sm.tile([H, NW], f32)
    nc.vector.tensor_copy(ssum[:], sq[0][:])
    for o in (-2, -1, 1, 2):
        t_m = sm.tile([H, NW], f32, tag=f"tm{o}")
        nc.vector.tensor_tensor(t_m[:], masks[o][:], sq[o][:], op=OP.mult)
        nc.vector.tensor_tensor(ssum[:], ssum[:], t_m[:], op=OP.add)
    # spix = cnt - ssum; build negated fp16 copy with a zero 97th row for the
    # -band/ch stationary
    spix = sm.tile([H, NW], f32)
    nc.vector.tensor_tensor(spix[:], cnt[:], ssum[:], op=OP.subtract)
    sneg = sm.tile([H + 1, NW], f16)
    nc.vector.memset(sneg[H:H + 1, :], 0.0)
    nc.vector.tensor_scalar(sneg[0:H, :], spix[:], -1.0, None, op0=OP.mult)
    ps_s = pmm2.tile([H, 512], f32, tag="ps2")
    nc.tensor.matmul(ps_s[:, 0:NW], bandch_sb[:], sneg[:], start=True, stop=True)
    sh = sm.tile([H, NW], f32)
    nc.scalar.copy(sh[:], ps_s[:, 0:NW])
    shp = sm.tile([H, NIMG, W + 4], f32)
    nc.vector.memset(shp[:], 0.0)
    for i in range(NIMG):
        nc.vector.tensor_copy(shp[:, i, 2:2 + W], sh[:, i * W:(i + 1) * W])
    swin = sm.tile([H, NIMG, W], f32)
    nc.vector.tensor_tensor(swin[:], shp[:, :, 0:W], shp[:, :, 1:1 + W], op=OP.add)
    for j in (2, 3, 4):
        nc.vector.tensor_tensor(swin[:], swin[:], shp[:, :, j:j + W], op=OP.add)
    sw_flat = swin[:].rearrange("p a b -> p (a b)")
    rinv = sm.tile([H, NW], f32)
    nc.vector.tensor_scalar(rinv[:], sw_flat, EPS, None, op0=OP.add)
    nc.vector.reciprocal(rinv[:], rinv[:])
    lnS = sm.tile([H, NW], f32)

    # =====================  main loop  =====================
    # Token tiles create artificial cross-batch deps so the Tile scheduler
    # cannot interleave Tanh and Ln activations (each interleave costs a
    # ~1.3us ACT table load): Ln(i) gates on last tanh(i) via its bias AP,
    # tanh(i+1) gates on last QL column of image i via its bias AP.
    QL = sm.tile([H, NW], f32)
    dummy = sm.tile([H, 1], f32)
    toks = {}

    def front(i):
        stripe = stripes[i % 2]
        t = None
        for g in range(NG):
            gi = i * NG + g
            ps1 = pmm1.tile([H, 1024], f32, tag="ps1")
            stat = st_sb[:, gi * H:(gi + 1) * H]
            nc.tensor.matmul(ps1[:, 0:512], stat, sel_sb[:, 0:512],
                             start=True, stop=True)
            nc.tensor.matmul(ps1[:, 512:1024], stat, sel_sb[:, 512:1024],
                             start=True, stop=True)
            t = tpool.tile([H, 1024], f16, tag="t")
            if i == 0:
                nc.scalar.activation(t[:], ps1[:], AF.Tanh)
            else:
                nc.scalar.activation(t[:], ps1[:], AF.Tanh, bias=toks[i][:])
            dst = stripe[0:H, (WG * g + 2) * NB:(WG * g + 2 + WG) * NB]
            nc.vector.tensor_tensor(dst, t[:], t[:], op=OP.mult)
        # eps token: value EPS, data-dependent on the last tanh of image i
        etok = sm.tile([H, 1], f32, tag=f"etok{i}")
        nc.vector.tensor_scalar(etok[:], t[:, 0:1], 0.0, EPS,
                                op0=OP.mult, op1=OP.add)
        return etok

    def back(i, etok):
        stripe = stripes[i % 2]
        if i == 0:
            nc.scalar.activation(lnS[:], sw_flat, AF.Ln, bias=etok[:])
        for c in range(W // 2):
            ps2 = pmm2.tile([H, 512], f32, tag="ps2")
            for tap, dw in enumerate((-2, -1, 0, 1, 2)):
                blk = 2 * c + 2 + dw
                nc.tensor.matmul(
                    ps2[:],
                    bandch_sb[:],
                    stripe[:, blk * NB:(blk + 2) * NB],
                    start=(tap == 0),
                    stop=(tap == 4),
                )
            L = lpool.tile([H, 512], f32, tag="L")
            nc.scalar.activation(L[:], ps2[:], AF.Ln, bias=etok[:])
            if BACKEND == "stt":
                for j in range(2):
                    w = 2 * c + j
                    nc.vector.scalar_tensor_tensor(
                        dummy.broadcast_to((H, NB)),
                        ps2[:, j * NB:(j + 1) * NB],
                        1.0,
                        L[:, j * NB:(j + 1) * NB],
                        op0=OP.mult,
                        op1=OP.mult,
                        accum_out=QL[:, i * W + w:i * W + w + 1],
                    )
            elif BACKEND == "ttr":
                for j in range(2):
                    w = 2 * c + j
                    nc.vector.tensor_tensor_reduce(
                        dummy.broadcast_to((H, NB)),
                        ps2[:, j * NB:(j + 1) * NB],
                        L[:, j * NB:(j + 1) * NB],
                        scale=1.0,
                        scalar=0.0,
                        op0=OP.mult,
                        op1=OP.add,
                        accum_out=QL[:, i * W + w:i * W + w + 1],
                    )
            else:
                w = 2 * c
                nc.vector.tensor_tensor(L[:], ps2[:], L[:], op=OP.mult)
                l3 = L[:].rearrange("p (a b) -> p a b", b=NB)
                nc.vector.tensor_reduce(
                    QL[:, i * W + w:i * W + w + 2],
                    l3,
                    axis=mybir.AxisListType.X,
                    op=OP.add,
                )
        # tanh token for image i+1: depends on last QL column of image i
        if i + 1 < NIMG:
            tok = sm.tile([H, 1], f32, tag=f"tok{i + 1}")
            nc.vector.tensor_scalar(
                tok[:], QL[:, (i + 1) * W - 1:(i + 1) * W], 0.0, None, op0=OP.mult
            )
            toks[i + 1] = tok

    for i in range(NIMG):
        etok = front(i)
        back(i, etok)

    # E = lnS - QL / S
    ent = sm.tile([H, NW], f32)
    nc.vector.tensor_tensor(ent[:], QL[:], rinv[:], op=OP.mult)
    nc.vector.tensor_tensor(ent[:], lnS[:], ent[:], op=OP.subtract)
    for i in range(NIMG):
        nc.sync.dma_start(ent_d[i], ent[:, i * W:(i + 1) * W])


def _get_compiled():
    if "nc" in _CACHE:
        return _CACHE["nc"]
    from contextlib import ExitStack

    import concourse.tile as tile
    from concourse import bacc, mybir

    f32 = mybir.dt.float32
    f16 = mybir.dt.float16
    nc = bacc.Bacc("TRN2", target_bir_lowering=False, debug=False)
    u_d = nc.dram_tensor("u_sh", [H, NW], f32, kind="ExternalInput").ap()
    nf_d = nc.dram_tensor("nf_sh", [H, NW], f32, kind="ExternalInput").ap()
    st_d = nc.dram_tensor("st_sh", [9, NIMG * NG * H], f16, kind="ExternalInput").ap()
    sel_d = nc.dram_tensor("sel", [9, WG * NB], f16, kind="ExternalInput").ap()
    bandch_d = nc.dram_tensor("bandch", [H + 1, H], f16, kind="ExternalInput").ap()
    padz_d = nc.dram_tensor("padz", [H + 1, 2 * NB], f16, kind="ExternalInput").ap()
    ones_d = nc.dram_tensor("onesmid", [1, W * NB], f16, kind="ExternalInput").ap()
    ent_d = nc.dram_tensor("ent", [NIMG, H, W], f32, kind="ExternalOutput").ap()

    with tile.TileContext(nc) as tc:
        with ExitStack() as ctx:
            _emit_kernel(
                nc, tc, ctx,
                (u_d, nf_d, st_d, sel_d, bandch_d, padz_d, ones_d),
                (ent_d,),
            )
    nc.compile()
    _CACHE["nc"] = nc
    return nc


def make_in_maps(x):
    """x: full [8, 3, 96, 96] -> list of 8 per-core input dicts."""
    x = np.ascontiguousarray(np.asarray(x, dtype=np.float32))
    imgs = x.reshape(NCORES * NIMG, H, W)
    sel, bandch = _build_consts()
    padz = np.zeros((H + 1, 2 * NB), dtype=np.float16)
    onesmid = np.ones((1, W * NB), dtype=np.float16)
    in_maps = []
    for c in range(NCORES):
        sh = imgs[c * NIMG:(c + 1) * NIMG]            # [3, 96, 96]
        n = np.trunc(sh)
        u = sh - n
        # [h, i*96+w] layouts for the S path
        u_all = np.ascontiguousarray(u.transpose(1, 0, 2).reshape(H, NW))
        nf_all = np.ascontiguousarray(n.transpose(1, 0, 2).reshape(H, NW))
        # stationary groups: [9, 3*24*96] fp16
        u5t = (5.0 * u).transpose(0, 2, 1).astype(np.float16)   # [3, 96w, 96h]
        n5t = (5.0 * n).transpose(0, 2, 1).astype(np.float16)
        st = np.empty((9, NIMG * NG * H), dtype=np.float16)
        st[8, :] = 1.0
        for i in range(NIMG):
            for g in range(NG):
                col = (i * NG + g) * H
                st[0:WG, col:col + H] = u5t[i, WG * g:WG * g + WG, :]
                st[WG:2 * WG, col:col + H] = n5t[i, WG * g:WG * g + WG, :]
        in_maps.append(
            {
                "u_sh": u_all,
                "nf_sh": nf_all,
                "st_sh": st,
                "sel": sel,
                "bandch": bandch,
                "padz": padz,
                "onesmid": onesmid,
            }
        )
    return in_maps


def kernel(x):
    """Full inputs in, full outputs out. x: [8, 3, 96, 96] f32."""
    from concourse.bass_utils import run_bass_kernel_spmd

    nc = _get_compiled()
    in_maps = make_in_maps(x)
    res = run_bass_kernel_spmd(nc, in_maps, list(range(NCORES)))
    out = np.stack([res.results[c]["ent"] for c in range(NCORES)])
    return out.reshape(8, 3, H, W).astype(np.float32)
